# revision 1
# baseline (speedup 1.0000x reference)
"""Trainium2 Bass kernel for nn_DifferentialGQA (8-core SPMD).

Strategy:
  - Projections + RoPE row-sharded: core c owns rows [256c, 256c+256).
  - One packed AllToAll reshards to pair-parallel attention: core c gets query
    heads 4c..4c+3 (= differential pairs 2c, 2c+1) and kv head c over the full
    sequence. Lambda partial dots ride along in the same collective.
  - Attention per (pair, qblock): f32r score matmuls (the pair's two heads run
    row-tiled on the two PE-array halves), tanh cap + exp on ACT (one table
    set) with free row-sum accumulation, diff = relu(e1 - lam*(r1/r2)*e2) on
    DVE (both softmax divisions fold into per-row scalars), PE-transposed
    diff -> PV.
  - RMS norm folds to one per-row rsqrt via ln+exp (one table switch, batched).
  - Second small AllToAll reshards back to rows; Wo (with subln, (1-lam0) and
    the duplicated-half pair-sum pre-folded on host) is a row-parallel f32r
    matmul. Host concatenates row slabs.
"""
import sys

sys.path.insert(0, "/opt/trn_rl_repo")

import numpy as np
import ml_dtypes

import concourse.bass as bass
import concourse.mybir as mybir
import concourse.tile as tile
from concourse import bacc
from concourse.bass_utils import run_bass_kernel_spmd
from concourse.masks import make_identity

dt = mybir.dt
AF = mybir.ActivationFunctionType
OP = mybir.AluOpType

N_CORES = 8
L = 2048
HID = 2048
H = 32
HKV = 8
D = 64
CAP = 50.0
LAMBDA_INIT = 0.8 - 0.6 * float(np.exp(-0.3 * 4))
P = 128
LROWS = L // N_CORES          # 256 rows per core
NQB = L // P                  # 16 query blocks
KT = HID // P                 # 16 contraction tiles
# packed A2A layout (uint16 cols): q 0:512, k 512:640, v 640:704, partials 704:712
A2A_COLS = 712


def _build(mock_collectives: bool = False, debug: bool = False):
    nc = bacc.Bacc("TRN2", target_bir_lowering=False, debug=False,
                   num_devices=(1 if mock_collectives else N_CORES))
    f32, f32r, bf16, u16 = dt.float32, dt.float32r, dt.bfloat16, dt.uint16

    xt = nc.dram_tensor("xt", [HID, LROWS], f32r, kind="ExternalInput").ap()
    wq = nc.dram_tensor("wq", [HID, HID], bf16, kind="ExternalInput").ap()
    wk = nc.dram_tensor("wk", [HID, HKV * D], f32r, kind="ExternalInput").ap()
    wv = nc.dram_tensor("wv", [HID, HKV * D], bf16, kind="ExternalInput").ap()
    wo = nc.dram_tensor("wo", [H * D // 2, HID], f32r, kind="ExternalInput").ap()
    cosr = nc.dram_tensor("cosr", [LROWS, H * 32], f32, kind="ExternalInput").ap()
    sinr = nc.dram_tensor("sinr", [LROWS, H * 32], f32, kind="ExternalInput").ap()
    coskr = nc.dram_tensor("coskr", [LROWS, HKV * 32], f32, kind="ExternalInput").ap()
    sinkr = nc.dram_tensor("sinkr", [LROWS, HKV * 32], f32, kind="ExternalInput").ap()
    wlq = nc.dram_tensor("wlq", [P, 2, HID], f32, kind="ExternalInput").ap()
    wlk = nc.dram_tensor("wlk", [P, 2, HKV * D], f32, kind="ExternalInput").ap()
    out_d = nc.dram_tensor("out", [LROWS, HID], f32, kind="ExternalOutput").ap()
    dbg = {}
    if debug:
        for nm, shp, dty in [
            ("d_q0", [P, HID], f32), ("d_k0", [P, HKV * D], f32),
            ("d_v0", [P, HKV * D], f32), ("d_pdots", [1, 4], f32),
            ("d_lam", [P, 1], f32), ("d_qT0", [P, L], f32),
            ("d_kT", [P, L], f32), ("d_vm", [P, D], f32),
            ("d_r1", [P, 32], f32), ("d_r2", [P, 32], f32),
            ("d_ssq", [P, 32], f32), ("d_scl", [P, 32], f32),
            ("d_out1", [P, NQB, P], f32), ("d_onT", [P, L], f32),
            ("d_e0", [P, L], f32), ("d_tanh0", [P, 2, L], f32),
        ]:
            dbg[nm] = nc.dram_tensor(nm, shp, dty, kind="ExternalOutput").ap()

    with tile.TileContext(nc) as tc:
        with (
            tc.tile_pool(name="persist", bufs=1) as pp,
            tc.tile_pool(name="dram", bufs=1, space="DRAM") as dram,
        ):
            ag_in = dram.tile([L, A2A_COLS], u16, tag="ag_in")
            ag_out = dram.tile([L, A2A_COLS], u16, tag="ag_out")
            a2_in = dram.tile([N_CORES * P, LROWS], f32r, tag="a2_in")
            a2_out = dram.tile([N_CORES * P, LROWS], f32r, tag="a2_out")

            ident_bf = pp.tile([P, P], bf16, tag="ident_bf")
            ident_f = pp.tile([P, P], f32, tag="ident_f")
            make_identity(nc, ident_bf[:])
            make_identity(nc, ident_f[:])
            ones_row = pp.tile([1, P], f32, tag="ones_row")
            nc.gpsimd.memset(ones_row[:], 1.0)
            ones_col = pp.tile([P, 1], f32, tag="ones_col")
            nc.gpsimd.memset(ones_col[:], 1.0)

            # persistent cross-phase tensors
            qTs = [pp.tile([P, L], f32r, tag=f"qT{i}", name=f"qT{i}") for i in range(2)]
            kT = pp.tile([P, L], f32r, tag="kT")      # kv head on both halves
            vm = pp.tile([P, NQB, D], bf16, tag="vm")  # v rows [l, d]
            lamneg_bc = pp.tile([P, 1], f32, tag="lamneg")
            rbuf1 = pp.tile([P, 32], f32, tag="rbuf1")
            rbuf2 = pp.tile([P, 32], f32, tag="rbuf2")
            ssqb = pp.tile([P, 32], f32, tag="ssqb")
            scl = pp.tile([P, 32], f32, tag="scl")
            out1_all = pp.tile([P, NQB, P], f32, tag="out1")  # [q, qb, 2x64]
            onT = pp.tile([P, L], f32r, tag="onT")            # out1nT [dcat, L]

            # ---------- Phase A: projections + rope + lambda partials ----------
            with (
                tc.tile_pool(name="pa", bufs=1) as pa,
                tc.tile_pool(name="pa2", bufs=2) as pa2,
            ):
                xt_sb = pa.tile([P, KT, LROWS], f32r, tag="xt")
                for kt in range(KT):
                    eng = nc.sync if kt % 2 == 0 else nc.scalar
                    eng.dma_start(xt_sb[:, kt, :], xt[kt * P:(kt + 1) * P, :])
                xt_bf = pa.tile([P, KT, LROWS], bf16, tag="xtbf")
                nc.vector.tensor_copy(xt_bf[:], xt_sb[:])

                q_sb = [pa.tile([P, HID], f32, tag=f"q_sb{lg}", name=f"q_sb{lg}") for lg in range(2)]
                k_sb = [pa.tile([P, HKV * D], f32, tag=f"k_sb{lg}", name=f"k_sb{lg}") for lg in range(2)]
                v_sb = [pa.tile([P, HKV * D], bf16, tag=f"v_sb{lg}", name=f"v_sb{lg}") for lg in range(2)]

                with tc.tile_pool(name="psA", bufs=1, space="PSUM") as psA:
                    # ---- q: both l-groups accumulate while Wq streams once ----
                    qps = [psA.tile([P, HID], f32, tag=f"ps_{lg}", name=f"qps{lg}") for lg in range(2)]
                    for kt in range(KT):
                        wq_t = pa2.tile([P, HID], bf16, tag="wq", bufs=3)
                        nc.sync.dma_start(wq_t[:], wq[kt * P:(kt + 1) * P, :])
                        for lg in range(2):
                            for n4 in range(4):
                                nc.tensor.matmul(
                                    qps[lg][:, n4 * 512:(n4 + 1) * 512],
                                    xt_bf[:, kt, lg * P:(lg + 1) * P],
                                    wq_t[:, n4 * 512:(n4 + 1) * 512],
                                    start=(kt == 0), stop=(kt == KT - 1))
                    for lg in range(2):
                        cr = pa2.tile([P, H * 32], f32, tag="cr")
                        sr = pa2.tile([P, H * 32], f32, tag="sr")
                        nc.scalar.dma_start(cr[:], cosr[lg * P:(lg + 1) * P, :])
                        nc.scalar.dma_start(sr[:], sinr[lg * P:(lg + 1) * P, :])
                        ta = pa2.tile([P, H * 32], f32, tag="ta")
                        tb = pa2.tile([P, H * 32], f32, tag="tb")
                        qp3 = qps[lg][:].rearrange("p (h j) -> p h j", j=D)
                        q3 = q_sb[lg][:].rearrange("p (h j) -> p h j", j=D)
                        c3 = cr[:].rearrange("p (h j) -> p h j", j=32)
                        s3 = sr[:].rearrange("p (h j) -> p h j", j=32)
                        ta3 = ta[:].rearrange("p (h j) -> p h j", j=32)
                        tb3 = tb[:].rearrange("p (h j) -> p h j", j=32)
                        nc.vector.tensor_tensor(ta3[:], qp3[:, :, 32:64], s3[:], OP.mult)
                        nc.vector.tensor_tensor(tb3[:], qp3[:, :, 0:32], s3[:], OP.mult)
                        nc.vector.tensor_tensor(
                            q3[:, :, 0:32], qp3[:, :, 0:32], c3[:], OP.mult)
                        nc.vector.tensor_tensor(
                            q3[:, :, 32:64], qp3[:, :, 32:64], c3[:], OP.mult)
                        nc.vector.tensor_tensor(
                            q3[:, :, 0:32], q3[:, :, 0:32], ta3[:], OP.subtract)
                        nc.vector.tensor_tensor(
                            q3[:, :, 32:64], q3[:, :, 32:64], tb3[:], OP.add)

                    # ---- k, v (reuse the q psum tags after rope drains) ----
                    kps = [psA.tile([P, HKV * D], f32, tag=f"ps_{lg}", name=f"kps{lg}") for lg in range(2)]
                    for kt in range(KT):
                        wk_t = pa2.tile([P, HKV * D], f32r, tag="wk")
                        nc.scalar.dma_start(wk_t[:], wk[kt * P:(kt + 1) * P, :])
                        for lg in range(2):
                            nc.tensor.matmul(
                                kps[lg][:], xt_sb[:, kt, lg * P:(lg + 1) * P],
                                wk_t[:], start=(kt == 0), stop=(kt == KT - 1))
                    for lg in range(2):
                        ckr = pa2.tile([P, HKV * 32], f32, tag="ckr")
                        skr = pa2.tile([P, HKV * 32], f32, tag="skr")
                        nc.scalar.dma_start(ckr[:], coskr[lg * P:(lg + 1) * P, :])
                        nc.scalar.dma_start(skr[:], sinkr[lg * P:(lg + 1) * P, :])
                        kta = pa2.tile([P, HKV * 32], f32, tag="kta")
                        ktb = pa2.tile([P, HKV * 32], f32, tag="ktb")
                        kp3 = kps[lg][:].rearrange("p (h j) -> p h j", j=D)
                        k3 = k_sb[lg][:].rearrange("p (h j) -> p h j", j=D)
                        kc3 = ckr[:].rearrange("p (h j) -> p h j", j=32)
                        ks3 = skr[:].rearrange("p (h j) -> p h j", j=32)
                        kta3 = kta[:].rearrange("p (h j) -> p h j", j=32)
                        ktb3 = ktb[:].rearrange("p (h j) -> p h j", j=32)
                        nc.vector.tensor_tensor(kta3[:], kp3[:, :, 32:64], ks3[:], OP.mult)
                        nc.vector.tensor_tensor(ktb3[:], kp3[:, :, 0:32], ks3[:], OP.mult)
                        nc.vector.tensor_tensor(
                            k3[:, :, 0:32], kp3[:, :, 0:32], kc3[:], OP.mult)
                        nc.vector.tensor_tensor(
                            k3[:, :, 32:64], kp3[:, :, 32:64], kc3[:], OP.mult)
                        nc.vector.tensor_tensor(
                            k3[:, :, 0:32], k3[:, :, 0:32], kta3[:], OP.subtract)
                        nc.vector.tensor_tensor(
                            k3[:, :, 32:64], k3[:, :, 32:64], ktb3[:], OP.add)
                    vps = [psA.tile([P, HKV * D], f32, tag=f"ps_{lg}", name=f"vps{lg}") for lg in range(2)]
                    for kt in range(KT):
                        wv_t = pa2.tile([P, HKV * D], bf16, tag="wv")
                        nc.scalar.dma_start(wv_t[:], wv[kt * P:(kt + 1) * P, :])
                        for lg in range(2):
                            nc.tensor.matmul(
                                vps[lg][:], xt_bf[:, kt, lg * P:(lg + 1) * P],
                                wv_t[:], start=(kt == 0), stop=(kt == KT - 1))
                    for lg in range(2):
                        nc.vector.tensor_copy(v_sb[lg][:], vps[lg][:])

                # ---- lambda partial dots (PSUM pools above are closed) ----
                wlq_sb = pa.tile([P, 2, HID], f32, tag="wlq")
                wlk_sb = pa.tile([P, 2, HKV * D], f32, tag="wlk")
                nc.scalar.dma_start(wlq_sb[:], wlq[:])
                nc.scalar.dma_start(wlk_sb[:], wlk[:])
                acc = pa.tile([P, 8], f32, tag="acc")  # col = dot*2 + lg
                scratch = pa.tile([P, HID], f32, tag="scratch")
                for lg in range(2):
                    for d_i in range(2):
                        nc.vector.scalar_tensor_tensor(
                            out=scratch[:], in0=q_sb[lg][:], scalar=1.0,
                            in1=wlq_sb[:, d_i, :], op0=OP.mult, op1=OP.mult,
                            accum_out=acc[:, (2 * d_i) * 2 + lg:(2 * d_i) * 2 + lg + 1])
                    for d_i in range(2):
                        nc.vector.scalar_tensor_tensor(
                            out=scratch[:, :HKV * D], in0=k_sb[lg][:], scalar=1.0,
                            in1=wlk_sb[:, d_i, :], op0=OP.mult, op1=OP.mult,
                            accum_out=acc[:, (2 * d_i + 1) * 2 + lg:(2 * d_i + 1) * 2 + lg + 1])
                pdots_sb = pa.tile([1, 4], f32, tag="pdots")
                with tc.tile_pool(name="psA2", bufs=1, space="PSUM") as psA2:
                    dots_ps = psA2.tile([1, 4], f32, tag="dots")
                    for d_i in range(4):
                        for lg in range(2):
                            nc.tensor.matmul(
                                dots_ps[:, d_i:d_i + 1], ones_col[:],
                                acc[:, d_i * 2 + lg:d_i * 2 + lg + 1],
                                start=(lg == 0), stop=(lg == 1))
                    nc.vector.tensor_copy(pdots_sb[:], dots_ps[:])

                # ---- write A2A contribution (2D HWDGE-friendly DMAs) ----
                for j in range(N_CORES):
                    e1, e2 = (nc.sync, nc.scalar) if j % 2 == 0 else (nc.scalar, nc.sync)
                    for lg in range(2):
                        rows = ag_in[j * LROWS + lg * P: j * LROWS + (lg + 1) * P, :]
                        e1.dma_start(
                            rows[:, 0:512].bitcast(f32),
                            q_sb[lg][:, j * 256:(j + 1) * 256])
                        e2.dma_start(
                            rows[:, 512:640].bitcast(f32),
                            k_sb[lg][:, j * D:(j + 1) * D])
                        e2.dma_start(
                            rows[:, 640:704].bitcast(bf16),
                            v_sb[lg][:, j * D:(j + 1) * D])
                    nc.gpsimd.dma_start(
                        ag_in[j * LROWS: j * LROWS + 1, 704:712].bitcast(f32),
                        pdots_sb[:])

                if debug:
                    nc.sync.dma_start(dbg["d_q0"][:], q_sb[0][:])
                    nc.sync.dma_start(dbg["d_k0"][:], k_sb[0][:])
                    dv0 = pa.tile([P, HKV * D], f32, tag="dv0")
                    nc.vector.tensor_copy(dv0[:], v_sb[0][:])
                    nc.sync.dma_start(dbg["d_v0"][:], dv0[:])
                    nc.sync.dma_start(dbg["d_pdots"][:], pdots_sb[:])

            # ---------------- collective #1 ----------------
            if mock_collectives:
                nc.sync.dma_start(ag_out[:], ag_in[:])
            else:
                nc.gpsimd.collective_compute(
                    "AllToAll", OP.bypass,
                    replica_groups=[list(range(N_CORES))],
                    ins=[ag_in.opt()], outs=[ag_out.opt()])

            # ---------- Phase B: unpack, transpose, lambda ----------
            with (
                tc.tile_pool(name="pb", bufs=1) as pb,
                tc.tile_pool(name="pb2", bufs=2) as pb2,
                tc.tile_pool(name="psB", bufs=2, space="PSUM") as psB,
            ):
                # lambda: gather partials [8 cores x 4] onto one partition
                g = pb.tile([1, 32], f32, tag="g")
                part_ap = ag_out[:].bitcast(f32).rearrange(
                    "(i r) c -> i r c", r=LROWS)[:, 0, 352:356]
                nc.sync.dma_start(
                    g[:].rearrange("o (i c) -> o i c", c=4), part_ap.unsqueeze(0))
                g16 = pb.tile([1, 16], f32, tag="g16")
                g8 = pb.tile([1, 8], f32, tag="g8")
                g4 = pb.tile([1, 4], f32, tag="g4")
                nc.vector.tensor_tensor(g16[:], g[:, 0:16], g[:, 16:32], OP.add)
                nc.vector.tensor_tensor(g8[:], g16[:, 0:8], g16[:, 8:16], OP.add)
                nc.vector.tensor_tensor(g4[:], g8[:, 0:4], g8[:, 4:8], OP.add)
                nc.vector.tensor_scalar(
                    out=g4[:], in0=g4[:], scalar1=1.0 / L, scalar2=10.0,
                    op0=OP.mult, op1=OP.min)
                nc.vector.tensor_scalar(
                    out=g4[:], in0=g4[:], scalar1=-10.0, scalar2=None, op0=OP.max)
                ex4 = pb.tile([1, 4], f32, tag="ex4")
                nc.scalar.activation(ex4[:], g4[:], AF.Exp)
                lam_a = pb.tile([1, 1], f32, tag="lam_a")
                lam_b = pb.tile([1, 1], f32, tag="lam_b")
                nc.vector.tensor_tensor(lam_a[:], ex4[:, 0:1], ex4[:, 1:2], OP.mult)
                nc.vector.tensor_tensor(lam_b[:], ex4[:, 2:3], ex4[:, 3:4], OP.mult)
                lam_t = pb.tile([1, 1], f32, tag="lam_t")
                nc.vector.tensor_tensor(lam_t[:], lam_a[:], lam_b[:], OP.subtract)
                nc.vector.tensor_scalar(
                    out=lam_t[:], in0=lam_t[:], scalar1=LAMBDA_INIT, scalar2=0.0,
                    op0=OP.add, op1=OP.max)
                nc.vector.tensor_scalar(
                    out=lam_t[:], in0=lam_t[:], scalar1=1.0, scalar2=-1.0,
                    op0=OP.min, op1=OP.mult)  # negated lambda
                lam_ps = psB.tile([P, 1], f32, tag="lam_ps")
                nc.tensor.matmul(lam_ps[:], ones_row[:], lam_t[:], start=True, stop=True)
                nc.vector.tensor_copy(lamneg_bc[:], lam_ps[:])

                # unpack + transpose q, k; load v
                agf = ag_out[:].bitcast(f32)  # [L, 356]
                qm = pb.tile([P, NQB, 256], f32, tag="qm")
                km = pb.tile([P, NQB, D], f32, tag="km")
                for lgrp in range(NQB):
                    eng = nc.sync if lgrp % 2 == 0 else nc.scalar
                    eng.dma_start(qm[:, lgrp, :], agf[lgrp * P:(lgrp + 1) * P, 0:256])
                    eng.dma_start(km[:, lgrp, :], agf[lgrp * P:(lgrp + 1) * P, 256:320])
                for lgrp in range(NQB):
                    for dg in range(2):
                        tq = psB.tile([P, P], f32, tag="tq")
                        nc.tensor.transpose(
                            tq[:], qm[:, lgrp, dg * P:(dg + 1) * P], ident_f[:])
                        nc.vector.tensor_copy(
                            qTs[dg][:, lgrp * P:(lgrp + 1) * P], tq[:])
                    tk = psB.tile([D, P], f32, tag="tk")
                    nc.tensor.transpose(tk[:], km[:, lgrp, :], ident_f[:])
                    nc.vector.tensor_copy(kT[0:D, lgrp * P:(lgrp + 1) * P], tk[:])
                agb = ag_out[:].bitcast(bf16)
                for b in range(NQB):
                    eng = nc.sync if b % 2 == 0 else nc.scalar
                    eng.dma_start(vm[:, b, :], agb[b * P:(b + 1) * P, 640:704])
                # duplicate kv head onto partitions 64:128 (DMA moves partitions);
                # per-block so s2 score matmuls unblock incrementally
                for b_ in range(NQB):
                    eng = nc.sync if b_ % 2 == 0 else nc.scalar
                    eng.dma_start(kT[D:2 * D, b_ * P:(b_ + 1) * P],
                                  kT[0:D, b_ * P:(b_ + 1) * P])

            # Wo prefetched early so its DMA overlaps attention
            with tc.tile_pool(name="pw", bufs=1) as pw:
                wo_sb = pw.tile([P, N_CORES, HID], f32r, tag="wo_sb")
                for d_ in range(N_CORES):
                    eng = nc.scalar if d_ % 2 == 0 else nc.sync
                    eng.dma_start(wo_sb[:, d_, :], wo[d_ * P:(d_ + 1) * P, :])

                # ---------------- Phase C: attention ----------------
                with (
                    tc.tile_pool(name="pc", bufs=1) as pc,
                    tc.tile_pool(name="pc2", bufs=2) as pc2,
                    tc.tile_pool(name="psC", bufs=1, space="PSUM") as psC,
                    tc.tile_pool(name="psC2", bufs=2, space="PSUM") as psC2,
                ):
                    sq_scr = pc.tile([P, D], f32, tag="sq_scr")
                    for qb in range(NQB):
                        for pair in range(2):
                            qTp = qTs[pair]
                            u = pair * NQB + qb
                            span = (qb + 1) * P
                            tanh_b = pc2.tile([P, 2, L], f32, tag="tanh")
                            for ch in range((span + 511) // 512):
                                c0 = ch * 512
                                csp = min(512, span - c0)
                                sps = psC2.tile([P, 2, 512], f32, tag="scores")
                                for t in range(2):
                                    nc.tensor.matmul(
                                        sps[:, t, 0:csp],
                                        qTp[t * D:(t + 1) * D, qb * P:(qb + 1) * P],
                                        kT[t * D:(t + 1) * D, c0:c0 + csp],
                                        start=True, stop=True)
                                nc.scalar.activation(
                                    tanh_b[:, :, c0:c0 + csp], sps[:, :, 0:csp],
                                    AF.Tanh, scale=1.0 / (CAP * float(np.sqrt(D))))
                            # causal mask on the diagonal block (both tensors)
                            nc.gpsimd.affine_select(
                                out=tanh_b[:, :, qb * P:(qb + 1) * P],
                                in_=tanh_b[:, :, qb * P:(qb + 1) * P],
                                compare_op=OP.is_ge, fill=-1e9, base=0,
                                pattern=[[0, 2], [-1, P]], channel_multiplier=1)
                            e_b = pc2.tile([P, 2, L], bf16, tag="e")
                            nc.scalar.activation(
                                e_b[:, 0, 0:span], tanh_b[:, 0, 0:span], AF.Exp,
                                scale=CAP, accum_out=rbuf1[:, u:u + 1])
                            nc.scalar.activation(
                                e_b[:, 1, 0:span], tanh_b[:, 1, 0:span], AF.Exp,
                                scale=CAP, accum_out=rbuf2[:, u:u + 1])
                            # lam' = -lam * r1 / r2   [128, 1]
                            if debug and pair == 0 and qb == 3:
                                de = pc2.tile([P, L], f32, tag="de")
                                nc.vector.tensor_copy(de[:, 0:span], e_b[:, 0, 0:span])
                                nc.sync.dma_start(dbg["d_e0"][:], de[:])
                                nc.sync.dma_start(dbg["d_tanh0"][:], tanh_b[:])
                            lam_p = pc2.tile([P, 1], f32, tag="lam_p")
                            nc.vector.reciprocal(lam_p[:], rbuf2[:, u:u + 1])
                            nc.vector.tensor_tensor(
                                lam_p[:], lam_p[:], rbuf1[:, u:u + 1], OP.mult)
                            nc.vector.tensor_tensor(
                                lam_p[:], lam_p[:], lamneg_bc[:], OP.mult)
                            diff = pc2.tile([P, L], bf16, tag="diff")
                            nc.vector.scalar_tensor_tensor(
                                out=diff[:, 0:span], in0=e_b[:, 1, 0:span],
                                scalar=lam_p[:], in1=e_b[:, 0, 0:span],
                                op0=OP.mult, op1=OP.add)
                            nc.vector.tensor_scalar(
                                out=diff[:, 0:span], in0=diff[:, 0:span],
                                scalar1=0.0, scalar2=None, op0=OP.max)
                            pv = psC2.tile([P, D], f32, tag="pv")
                            nkb = qb + 1
                            for grp in range((nkb + 3) // 4):
                                kb0 = grp * 4
                                ng = min(4, nkb - kb0)
                                trp = psC2.tile([P, 512], bf16, tag="tr")
                                for i in range(ng):
                                    nc.tensor.transpose(
                                        trp[:, i * P:(i + 1) * P],
                                        diff[:, (kb0 + i) * P:(kb0 + i + 1) * P],
                                        ident_bf[:])
                                dT = pc2.tile([P, 512], bf16, tag="dT")
                                nc.vector.tensor_copy(dT[:, 0:ng * P], trp[:, 0:ng * P])
                                for i in range(ng):
                                    kb = kb0 + i
                                    nc.tensor.matmul(
                                        pv[:], dT[:, i * P:(i + 1) * P], vm[:, kb, :],
                                        start=(kb == 0), stop=(kb == nkb - 1))
                            # stash out1, then ssq from the SBUF copy
                            o1 = out1_all[:, qb, pair * D:(pair + 1) * D]
                            nc.vector.tensor_copy(o1, pv[:])
                            nc.vector.scalar_tensor_tensor(
                                out=sq_scr[:], in0=o1, scalar=1.0, in1=o1,
                                op0=OP.mult, op1=OP.mult,
                                accum_out=ssqb[:, u:u + 1])

                # ---------------- Phase D: rms scale, repack, Wo ----------------
                with (
                    tc.tile_pool(name="pd", bufs=1) as pd,
                    tc.tile_pool(name="pd2", bufs=2) as pd2,
                ):
                    # scale = rsqrt(ssq/64 + 1e-6*r1^2)
                    rsq = pd.tile([P, 32], f32, tag="rsq")
                    nc.vector.tensor_tensor(rsq[:], rbuf1[:], rbuf1[:], OP.mult)
                    uarg = pd.tile([P, 32], f32, tag="uarg")
                    nc.vector.scalar_tensor_tensor(
                        out=uarg[:], in0=rsq[:], scalar=float(D) * 1e-6, in1=ssqb[:],
                        op0=OP.mult, op1=OP.add)
                    lnu = pd.tile([P, 32], f32, tag="lnu")
                    nc.scalar.activation(lnu[:], uarg[:], AF.Ln, scale=1.0 / D)
                    nc.scalar.activation(scl[:], lnu[:], AF.Exp, scale=-0.5)

                    out1n = pd.tile([P, NQB, P], f32, tag="out1n")
                    for pair in range(2):
                        for qb in range(NQB):
                            u = pair * NQB + qb
                            nc.vector.tensor_scalar(
                                out=out1n[:, qb, pair * D:(pair + 1) * D],
                                in0=out1_all[:, qb, pair * D:(pair + 1) * D],
                                scalar1=scl[:, u:u + 1], scalar2=None, op0=OP.mult)
                    with tc.tile_pool(name="psD", bufs=2, space="PSUM") as psD:
                        for qb in range(NQB):
                            t_ps = psD.tile([P, P], f32, tag="t_ps")
                            nc.tensor.transpose(t_ps[:], out1n[:, qb, :], ident_f[:])
                            nc.vector.tensor_copy(onT[:, qb * P:(qb + 1) * P], t_ps[:])
                    for j in range(N_CORES):
                        eng = nc.sync if j % 2 == 0 else nc.scalar
                        eng.dma_start(
                            a2_in[j * P:(j + 1) * P, :],
                            onT[:, j * LROWS:(j + 1) * LROWS])

                    if mock_collectives:
                        nc.sync.dma_start(a2_out[:], a2_in[:])
                    else:
                        nc.gpsimd.collective_compute(
                            "AllToAll", OP.bypass,
                            replica_groups=[list(range(N_CORES))],
                            ins=[a2_in.opt()], outs=[a2_out.opt()])

                    omT = pd.tile([P, N_CORES, LROWS], f32r, tag="omT")
                    for i in range(N_CORES):
                        eng = nc.sync if i % 2 == 0 else nc.scalar
                        eng.dma_start(omT[:, i, :], a2_out[i * P:(i + 1) * P, :])
                    with tc.tile_pool(name="psD2", bufs=1, space="PSUM") as psD2:
                        for lg in range(2):
                            ops = psD2.tile([P, HID], f32, tag=f"ops{lg}")
                            for dchunk in range(N_CORES):
                                for n4 in range(4):
                                    nc.tensor.matmul(
                                        ops[:, n4 * 512:(n4 + 1) * 512],
                                        omT[:, dchunk, lg * P:(lg + 1) * P],
                                        wo_sb[:, dchunk, n4 * 512:(n4 + 1) * 512],
                                        start=(dchunk == 0), stop=(dchunk == N_CORES - 1))
                            o_sb = pd.tile([P, HID], f32, tag=f"o_sb{lg}")
                            nc.vector.tensor_copy(o_sb[:], ops[:])
                            nc.sync.dma_start(out_d[lg * P:(lg + 1) * P, :], o_sb[:])

                    if debug:
                        nc.sync.dma_start(dbg["d_lam"][:], lamneg_bc[:])
                        dqt = pd.tile([P, L], f32, tag="dqt")
                        nc.vector.tensor_copy(dqt[:], qTs[0][:])
                        nc.sync.dma_start(dbg["d_qT0"][:], dqt[:])
                        nc.vector.tensor_copy(dqt[:], kT[:])
                        nc.sync.dma_start(dbg["d_kT"][:], dqt[:])
                        dvm = pd.tile([P, D], f32, tag="dvm")
                        nc.vector.tensor_copy(dvm[:], vm[:, 3, :])
                        nc.sync.dma_start(dbg["d_vm"][:], dvm[:])
                        nc.sync.dma_start(dbg["d_r1"][:], rbuf1[:])
                        nc.sync.dma_start(dbg["d_r2"][:], rbuf2[:])
                        nc.sync.dma_start(dbg["d_ssq"][:], ssqb[:])
                        nc.sync.dma_start(dbg["d_scl"][:], scl[:])
                        nc.sync.dma_start(dbg["d_out1"][:], out1_all[:])
                        nc.vector.tensor_copy(dqt[:], onT[:])
                        nc.sync.dma_start(dbg["d_onT"][:], dqt[:])

    return nc


_CACHE = {}


def _get_program():
    if "nc" not in _CACHE:
        nc = _build()
        nc.compile()
        _CACHE["nc"] = nc
    return _CACHE["nc"]


def _host_prep(x, cos, sin, Wq, Wk, Wv, Wo, lambda_q1, lambda_k1, lambda_q2,
               lambda_k2, subln_weight):
    x2 = np.asarray(x, np.float32).reshape(L, HID)
    xT = np.ascontiguousarray(x2.T)
    cos = np.asarray(cos, np.float32)[:L, :D // 2]
    sin = np.asarray(sin, np.float32)[:L, :D // 2]
    cosr = np.tile(cos, (1, H))                          # [L, 1024]
    sinr = np.tile(sin, (1, H))
    coskr = np.tile(cos, (1, HKV))                       # [L, 256]
    sinkr = np.tile(sin, (1, HKV))
    wlq = np.zeros((2, HID), np.float32)
    for h in range(H):
        if h % 2 == 0:
            wlq[0, h * D:(h + 1) * D] = np.asarray(lambda_q1, np.float32)
        else:
            wlq[1, h * D:(h + 1) * D] = np.asarray(lambda_q2, np.float32)
    wlk = np.zeros((2, HKV * D), np.float32)
    for h in range(HKV):
        wlk[0, h * D:(h + 1) * D] = 2.0 * np.asarray(lambda_k1, np.float32)
        wlk[1, h * D:(h + 1) * D] = 2.0 * np.asarray(lambda_k2, np.float32)
    wlq_rep = np.ascontiguousarray(np.broadcast_to(wlq[None], (P, 2, HID)))
    wlk_rep = np.ascontiguousarray(np.broadcast_to(wlk[None], (P, 2, HKV * D)))
    s = np.asarray(subln_weight, np.float32) * (1.0 - LAMBDA_INIT)   # [128]
    Wo = np.asarray(Wo, np.float32)
    wo_eff = np.empty((H * D // 2, HID), np.float32)
    for p in range(H // 2):
        blk = Wo[p * 2 * D:(p + 1) * 2 * D, :]           # [128, HID]
        wo_eff[p * D:(p + 1) * D] = (s[:D, None] * blk[:D] + s[D:, None] * blk[D:])
    wv_bf = np.asarray(Wv, np.float32).astype(ml_dtypes.bfloat16)
    common = {
        "wq": np.asarray(Wq, np.float32).astype(ml_dtypes.bfloat16), "wk": np.asarray(Wk, np.float32),
        "wv": wv_bf, "wo": wo_eff, "wlq": wlq_rep, "wlk": wlk_rep,
    }
    in_maps = []
    for c in range(N_CORES):
        r0, r1 = c * LROWS, (c + 1) * LROWS
        m = dict(common)
        m["xt"] = np.ascontiguousarray(xT[:, r0:r1])
        m["cosr"] = np.ascontiguousarray(cosr[r0:r1])
        m["sinr"] = np.ascontiguousarray(sinr[r0:r1])
        m["coskr"] = np.ascontiguousarray(coskr[r0:r1])
        m["sinkr"] = np.ascontiguousarray(sinkr[r0:r1])
        in_maps.append(m)
    return in_maps


def kernel(**inputs) -> np.ndarray:
    nc = _get_program()
    in_maps = _host_prep(**{k: v for k, v in inputs.items() if k != "mask"})
    res = run_bass_kernel_spmd(nc, in_maps, list(range(N_CORES)))
    out = np.concatenate([res.results[c]["out"] for c in range(N_CORES)], axis=0)
    return out.reshape(1, L, HID).astype(np.float32)

